# revision 11
# baseline (speedup 1.0000x reference)
"""Trainium2 Bass kernel for the CNF reversible backward solve.

Architecture (v7): the wall-clock is bound by the serial per-step chain
  tanh_even -> Z-matmuls -> tanh_odd -> V-matmuls
so both state banks are eternal PSUM accumulators and both tanh evals run
as ONE custom DVE instruction each (clamped odd-quintic minimax fit of
tanh, max pointwise err 1.4e-2, end-to-end rel err ~6e-3 -- validated in
fp64 simulation against the reference):

  out = clamp(x*(C0 + imm2*x^2 + Src1*x^4), -C1, +C1)

The Y-carry is eliminated by the substitution V_s = l^s * Yhat_s, making V
a pure accumulator:
  V_{s+1} = V_s + l^s*Mz a_o(s) + l^s(l-1)*Zb_{s+1} + l^s*D(s)
with a_e(s) = tanh(inv_l^{s+1} V_s) (input scale folded into the DVE-op
coefficients). The l^s factors go into 8 epoch-relative weight copies
(mztV[m] = l^m Mz, m = s mod 8) plus an in-place DVE rescale of the V bank
by inv_l^8 every 8 steps. The Zb coupling rides q = kappa_m*Zb computed on
the otherwise-idle Scalar engine (Copy-activation with scale) and injected
via an identity matmul; per-step bias deltas use baseline-style rank-2
bf16 matmuls.

Sharding: data-parallel, B=256 -> 32 samples on each of 8 cores.
Activations stream to DRAM; outputs are assembled host-side from
gamma-weighted activation sums (exact coefficient recursions in fp64).
"""

import numpy as np
import ml_dtypes
from contextlib import ExitStack

import concourse.bass as bass
import concourse.tile as tile
from concourse import bacc, mybir
from concourse import dve_ops as _dops
from concourse.dve_ops import DveOp
from concourse.dve_spec import (
    Spec, Src0, Src1, C0, C1, C2, Zero, minn, maxx, sq, lower, _has_src1,
)
from concourse.dve_uop import DveOpSpec
from concourse.bass_utils import run_bass_kernel_spmd

# Problem constants (hardcoded per contract)
NCORES = 8
B, D, H = 256, 64, 256
NSTEP = 64
HSTEP = 1.0 / NSTEP
LCOUP = 0.999
INVL = 1.0 / LCOUP
BS = B // NCORES  # 32 samples per core
NBLK = H // 128  # 2 h-blocks
FREE = NBLK * BS  # 64: free size of H-space tiles, layout (blk, sample)
EPOCH = 32
DMA_CHUNKS = 8
CSTEPS = NSTEP // DMA_CHUNKS  # 16 steps per out-DMA chunk
CCOLS = CSTEPS * FREE
ACOLS = NSTEP * FREE

F32 = mybir.dt.float32
BF16 = mybir.dt.bfloat16
BF16NP = ml_dtypes.bfloat16

# clamped odd-quintic minimax fit of tanh on [0, 6]
A0 = 0.9535417
A1 = -0.20116429
A2 = 0.02062697
CC = 0.98560722

SHARED_INPUTS = [
    "w1tb", "mzt", "mztv", "mztq", "ib16", "dbyv", "f32blob", "bf16blob",
    "a2tab",
]


# --- custom DVE op ---------------------------------------------------------

def _tanhq_ref(in0, in1, s0, s1, imm2):
    x = in0.astype(np.float32)
    u = x * x
    r = x * (np.float32(s0) + np.float32(imm2) * u + in1.astype(np.float32) * u * u)
    return np.clip(r, -np.float32(s1), np.float32(s1))


def _register_tanhq():
    name = "TANH_POLY_ANT"
    if name in _dops._SUB_OPCODE_FOR_NAME:
        return next(op for op in _dops.OPS if op.name == name)
    u = sq(Src0)
    p = (u * Src1 + C2) * u + C0
    body = maxx(minn(p * Src0, C1), Zero - C1)
    spec = Spec(body=body, reference=_tanhq_ref)
    uops = lower(spec, ver="v3")
    sha = DveOpSpec(name=name, opcode=0, uops=uops, rd1_en=_has_src1(spec)).sha("v3")
    op = DveOp(name, spec, subdim=False, uops_sha={"v3": sha})
    row = max(_dops._SUB_OPCODE_FOR_NAME.values()) + 1
    assert row < 0x20
    _dops.OPS.append(op)
    _dops._SUB_OPCODE_FOR_NAME[name] = row
    _dops.CUSTOM_DVE_SPECS[name] = op.spec
    return op


TANHQ = _register_tanhq()


def np_tanhq(x):
    x = np.asarray(x, dtype=np.float64)
    r = x * (A0 + A1 * x * x + A2 * x ** 4)
    return np.clip(r, -CC, CC)


# --- host-side tables ------------------------------------------------------

def _coefficients():
    """Exact fp64 scalar recursions for the output-extraction weights
    (identical to the reference recursion; device streams a_e, a_o)."""
    gamma = np.zeros(2 * NSTEP)
    la = np.zeros(2 * NSTEP)
    alpha_y = alpha_z = 1.0
    nu_y = nu_z = 0.0
    for s in range(NSTEP):
        la[2 * s] += -HSTEP
        nu_z += -HSTEP
        gamma *= INVL
        alpha_y *= INVL
        nu_y *= INVL
        gamma += (1.0 - INVL) * la
        alpha_y += (1.0 - INVL) * alpha_z
        nu_y += (1.0 - INVL) * nu_z
        gamma[2 * s + 1] += -INVL * HSTEP
        nu_y += -INVL * HSTEP
    return gamma, alpha_y, nu_y


def _host_tables(W1, b1, u1, W2, b2):
    W1 = W1.astype(np.float64)
    W2 = W2.astype(np.float64)
    b1 = b1.astype(np.float64)
    u1 = u1.astype(np.float64)
    b2 = b2.astype(np.float64)
    l = LCOUP

    Mz = -HSTEP * (W1 @ W2)  # [H, H]
    W1b2 = W1 @ b2

    def be(s):
        return b1 + (1.0 - s * HSTEP) * u1

    def bp(s):
        return b1 + (1.0 - (s + 1) * HSTEP) * u1 - (s + 1) * HSTEP * W1b2

    # mzt_pack[p, (k*NBLK+j)*128 + q] = Mz[128j+q, 128k+p]
    MzT = Mz.T
    def pack(M):
        out = np.zeros((128, NBLK * NBLK * 128))
        for k in range(NBLK):
            for j in range(NBLK):
                out[:, (k * NBLK + j) * 128 : (k * NBLK + j + 1) * 128] = M[
                    128 * k : 128 * k + 128, 128 * j : 128 * j + 128
                ]
        return out

    mzt_pack = pack(MzT)
    mztv = np.concatenate([(l ** m) * mzt_pack for m in range(EPOCH)], axis=1)
    mztq = (l - 1.0) * mztv

    # rank-2 bias tables
    dzc = -HSTEP * u1 - HSTEP * W1b2  # constant Zb delta (s>=1)
    dzcb = np.zeros((2, 128))
    for k in range(NBLK):
        dzcb[k, :] = dzc[128 * k : 128 * k + 128]

    dzc_ = dzc
    dbyv = np.zeros((2, NSTEP * 128))
    for s in range(NSTEP - 1):
        m = s % EPOCH
        Ds = -be(s) - (l - 1.0) * bp(s) - HSTEP * W1b2 + l * be(s + 1)
        v = (l ** m) * Ds
        if s >= 1:
            v = v + (l ** m) * (l - 1.0) * dzc_
        for k in range(NBLK):
            dbyv[k, s * 128 : (s + 1) * 128] = v[128 * k : 128 * k + 128]

    # init biases (fp32 rank-2)
    zb0 = np.zeros((2, 128))
    vb0 = np.zeros((2, 128))
    for k in range(NBLK):
        zb0[k, :] = bp(0)[128 * k : 128 * k + 128]
        vb0[k, :] = (l * be(0))[128 * k : 128 * k + 128]

    ind = np.zeros((2, FREE))
    for k in range(NBLK):
        ind[k, k * BS : (k + 1) * BS] = 1.0

    # a2 coefficient tiles (full-width: the [P,1]-broadcast in1 mode is
    # broken on HW): block m in [0,EPOCH) = even-eval A2*sigma_m^5, block
    # EPOCH = odd-eval plain A2
    a2tab = np.zeros((128, (EPOCH + 1) * FREE))
    for m in range(EPOCH):
        a2tab[:, m * FREE : (m + 1) * FREE] = A2 * (INVL ** (m + 1)) ** 5
    a2tab[:, EPOCH * FREE :] = A2

    f32blob = np.concatenate([zb0, vb0, ind], axis=1)  # [2, 256+64]
    bf16blob = np.concatenate([dzcb, ind], axis=1)  # [2, 128+64]
    w1tb = np.concatenate([W1.T, l * W1.T], axis=1)  # [64, 512]
    return dict(
        mzt=mzt_pack.astype(BF16NP),
        mztv=mztv.astype(BF16NP),
        mztq=mztq.astype(BF16NP),
        ib16=np.eye(128).astype(BF16NP),
        dbyv=dbyv.astype(BF16NP),
        f32blob=f32blob.astype(np.float32),
        bf16blob=bf16blob.astype(BF16NP),
        a2tab=a2tab.astype(np.float32),
        w1tb=w1tb.astype(np.float32),
    )


# --- device kernel ---------------------------------------------------------

def _build_kernel():
    nc = bacc.Bacc("TRN2", target_bir_lowering=False, debug=False)

    y1t_d = nc.dram_tensor("y1t", [D, BS], F32, kind="ExternalInput").ap()
    w1tb_d = nc.dram_tensor("w1tb", [D, 2 * H], F32, kind="ExternalInput").ap()
    mzt_d = nc.dram_tensor("mzt", [128, NBLK * NBLK * 128], BF16, kind="ExternalInput").ap()
    mztv_d = nc.dram_tensor("mztv", [128, EPOCH * NBLK * NBLK * 128], BF16, kind="ExternalInput").ap()
    mztq_d = nc.dram_tensor("mztq", [128, EPOCH * NBLK * NBLK * 128], BF16, kind="ExternalInput").ap()
    ib16_d = nc.dram_tensor("ib16", [128, 128], BF16, kind="ExternalInput").ap()
    dbyv_d = nc.dram_tensor("dbyv", [2, NSTEP * 128], BF16, kind="ExternalInput").ap()
    f32blob_d = nc.dram_tensor("f32blob", [2, 256 + FREE], F32, kind="ExternalInput").ap()
    bf16blob_d = nc.dram_tensor("bf16blob", [2, 128 + FREE], BF16, kind="ExternalInput").ap()
    a2tab_d = nc.dram_tensor("a2tab", [128, (EPOCH + 1) * FREE], F32, kind="ExternalInput").ap()

    ae_out_d = nc.dram_tensor("ae_out", [128, ACOLS], BF16, kind="ExternalOutput").ap()
    ao_out_d = nc.dram_tensor("ao_out", [128, ACOLS], BF16, kind="ExternalOutput").ap()

    with tile.TileContext(nc) as tc, ExitStack() as ctx:
        consts = ctx.enter_context(tc.tile_pool(name="consts", bufs=1))
        zpool = ctx.enter_context(tc.tile_pool(name="zps", bufs=1, space="PSUM"))
        vpool = ctx.enter_context(tc.tile_pool(name="vps", bufs=1, space="PSUM"))
        qpool = ctx.enter_context(tc.tile_pool(name="qtmp", bufs=2))

        def cload(name, shape, dt, dram):
            t = consts.tile(shape, dt, tag=name, name=name)
            nc.sync.dma_start(t[:], dram)
            return t

        # init-critical loads split across two DMA queues (sync + gpsimd)
        y1t = consts.tile([D, BS], F32, tag="y1t", name="y1t")
        nc.sync.dma_start(y1t[:], y1t_d)
        w1tb = consts.tile([D, 2 * H], F32, tag="w1tb", name="w1tb")
        nc.gpsimd.dma_start(w1tb[:], w1tb_d)
        f32blob = consts.tile([2, 256 + FREE], F32, tag="f32blob", name="f32blob")
        nc.sync.dma_start(f32blob[:], f32blob_d)
        a2tab = consts.tile([128, (EPOCH + 1) * FREE], F32, tag="a2tab", name="a2tab")
        nc.gpsimd.dma_start(a2tab[:], a2tab_d)
        bf16blob = consts.tile([2, 128 + FREE], BF16, tag="bf16blob", name="bf16blob")
        nc.sync.dma_start(bf16blob[:], bf16blob_d)
        mzt = consts.tile([128, NBLK * NBLK * 128], BF16, tag="mzt", name="mzt")
        nc.gpsimd.dma_start(mzt[:], mzt_d)
        ib16 = consts.tile([128, 128], BF16, tag="ib16", name="ib16")
        nc.sync.dma_start(ib16[:], ib16_d)
        dbyv = consts.tile([2, NSTEP * 128], BF16, tag="dbyv", name="dbyv")
        nc.gpsimd.dma_start(dbyv[:], dbyv_d)

        # big per-epoch weight packs streamed in epoch order (epoch m is
        # first needed at step 8m)
        EB = NBLK * NBLK * 128
        mztv = consts.tile([128, EPOCH * EB], BF16, tag="mztv", name="mztv")
        mztq = consts.tile([128, EPOCH * EB], BF16, tag="mztq", name="mztq")
        for m in range(EPOCH):
            nc.sync.dma_start(mztv[:, m * EB : (m + 1) * EB], mztv_d[:, m * EB : (m + 1) * EB])
            nc.gpsimd.dma_start(mztq[:, m * EB : (m + 1) * EB], mztq_d[:, m * EB : (m + 1) * EB])

        abuf_e = [
            consts.tile([128, CCOLS], BF16, tag=f"abe{c}", name=f"abe{c}")
            for c in range(DMA_CHUNKS)
        ]
        abuf_o = [
            consts.tile([128, CCOLS], BF16, tag=f"abo{c}", name=f"abo{c}")
            for c in range(DMA_CHUNKS)
        ]

        def mzt_blk(k, j):
            base = (k * NBLK + j) * 128
            return mzt[:, base : base + 128]

        def mztv_blk(m, k, j):
            base = (m * NBLK * NBLK + k * NBLK + j) * 128
            return mztv[:, base : base + 128]

        def mztq_blk(m, k, j):
            base = (m * NBLK * NBLK + k * NBLK + j) * 128
            return mztq[:, base : base + 128]

        # --- init: Zb = W1 y + bp(0); V = l*(W1 y + be(0)) ---
        zb = zpool.tile([128, FREE], F32, tag="zb", name="zb")
        for j in range(NBLK):
            nc.tensor.matmul(
                zb[:, j * BS : (j + 1) * BS],
                w1tb[:, 128 * j : 128 * j + 128],
                y1t[:],
                start=(j == 0),
                stop=False,
            )
        nc.tensor.matmul(zb[:], f32blob[:, 0:128], f32blob[:, 256 : 256 + FREE], start=False, stop=True)

        vb = vpool.tile([128, FREE], F32, tag="vb", name="vb")
        for j in range(NBLK):
            nc.tensor.matmul(
                vb[:, j * BS : (j + 1) * BS],
                w1tb[:, H + 128 * j : H + 128 * j + 128],
                y1t[:],
                start=(j == 0),
                stop=False,
            )
        nc.tensor.matmul(vb[:], f32blob[:, 128:256], f32blob[:, 256 : 256 + FREE], start=False, stop=True)

        for s in range(NSTEP):
            last = s == NSTEP - 1
            m = s % EPOCH
            chunk, cstep = divmod(s, CSTEPS)
            ecol = cstep * FREE
            sigma = INVL ** (m + 1)

            # q' = kappa_m * Zb_pre on the scalar engine (pre-step Zb state;
            # runs in the tanh-even window, different PSUM bank)
            if not last:
                q_t = qpool.tile([128, FREE], BF16, tag="q", name=f"q{s}")
                nc.scalar.activation(
                    q_t[:], zb[:], mybir.ActivationFunctionType.Copy,
                    scale=float((LCOUP ** m) * (LCOUP - 1.0)),
                )

            # even eval: a_e = tanhq(sigma * V)
            a_e = abuf_e[chunk][:, ecol : ecol + FREE]
            nc.vector._custom_dve(
                TANHQ, out=a_e, in0=vb[:], in1=a2tab[:, m * FREE : (m + 1) * FREE],
                s0=A0 * sigma, s1=CC, imm2=A1 * sigma ** 3,
            )

            # Zb += dzc (s>=1) + Mz a_e
            if s > 0:
                nc.tensor.matmul(zb[:], bf16blob[:, 0:128], bf16blob[:, 128 : 128 + FREE], start=False,
                                 stop=False, skip_group_check=True)
            for j in range(NBLK):
                for k in range(NBLK):
                    nc.tensor.matmul(
                        zb[:, j * BS : (j + 1) * BS],
                        mzt_blk(k, j),
                        a_e[:, k * BS : (k + 1) * BS],
                        start=False,
                        stop=False,
                        skip_group_check=True,
                    )

            if not last:
                # a_e-dependent V terms (run during tanh-odd): dbyv const,
                # q' inject, kappa_m*Mz @ a_e
                nc.tensor.matmul(vb[:], dbyv[:, s * 128 : (s + 1) * 128], bf16blob[:, 128 : 128 + FREE],
                                 start=False, stop=False, skip_group_check=True)
                nc.tensor.matmul(vb[:], ib16[:], q_t[:], start=False,
                                 stop=False, skip_group_check=True)
                for j in range(NBLK):
                    for k in range(NBLK):
                        nc.tensor.matmul(
                            vb[:, j * BS : (j + 1) * BS],
                            mztq_blk(m, k, j),
                            a_e[:, k * BS : (k + 1) * BS],
                            start=False,
                            stop=False,
                            skip_group_check=True,
                        )

            # odd eval: a_o = tanhq(Zb)
            a_o = abuf_o[chunk][:, ecol : ecol + FREE]
            nc.vector._custom_dve(
                TANHQ, out=a_o, in0=zb[:],
                in1=a2tab[:, EPOCH * FREE : (EPOCH + 1) * FREE],
                s0=A0, s1=CC, imm2=A1,
            )

            if not last:
                # V += l^m Mz a_o
                for j in range(NBLK):
                    for k in range(NBLK):
                        nc.tensor.matmul(
                            vb[:, j * BS : (j + 1) * BS],
                            mztv_blk(m, k, j),
                            a_o[:, k * BS : (k + 1) * BS],
                            start=False,
                            stop=False,
                            skip_group_check=True,
                        )

                if m == EPOCH - 1:
                    # epoch boundary: V *= inv_l^8 in place
                    nc.vector.tensor_scalar_mul(vb[:], vb[:], float(INVL ** EPOCH))

            if (s + 1) % CSTEPS == 0:
                c0 = chunk * CCOLS
                nc.sync.dma_start(ae_out_d[:, c0 : c0 + CCOLS], abuf_e[chunk][:])
                nc.sync.dma_start(ao_out_d[:, c0 : c0 + CCOLS], abuf_o[chunk][:])

    nc.compile()
    return nc


_CACHE = {}


def _get_kernel():
    if "nc" not in _CACHE:
        _CACHE["nc"] = _build_kernel()
    return _CACHE["nc"]


def kernel(y1, W1, b1, u1, W2, b2, _trace=False, _trace_kwargs=None):
    y1 = np.asarray(y1)
    in_dtype = y1.dtype
    W1_ = np.asarray(W1, dtype=np.float64)
    W2_ = np.asarray(W2, dtype=np.float64)
    b2_ = np.asarray(b2, dtype=np.float64)
    tabs = _host_tables(
        np.asarray(W1), np.asarray(b1), np.asarray(u1), np.asarray(W2), np.asarray(b2)
    )

    nc = _get_kernel()

    shared = {k: tabs[k] for k in SHARED_INPUTS}
    in_maps = []
    for c in range(NCORES):
        mmap = dict(shared)
        shard = y1[c * BS : (c + 1) * BS].astype(np.float32)  # [BS, D]
        mmap["y1t"] = np.ascontiguousarray(shard.T)  # [D, BS]
        in_maps.append(mmap)

    kw = {}
    if _trace:
        kw["trace"] = True
        if _trace_kwargs:
            kw.update(_trace_kwargs)
    res = run_bass_kernel_spmd(nc, in_maps, core_ids=list(range(NCORES)), **kw)

    # --- host-side output extraction ---
    gamma, c_y, c_b = _coefficients()
    cvec = np.sum(W1_ * W2_.T, axis=1)  # diag(W1@W2)
    sum_c = float(np.sum(cvec))

    out = np.zeros((B, D + 1), dtype=np.float32)
    for c in range(NCORES):
        ae = np.asarray(res.results[c]["ae_out"]).astype(np.float64)
        ao = np.asarray(res.results[c]["ao_out"]).astype(np.float64)
        ae = ae.reshape(128, NSTEP, NBLK, BS)  # [p, s, blk, b]
        ao = ao.reshape(128, NSTEP, NBLK, BS)
        ae = np.moveaxis(ae, (2, 0), (1, 2)).reshape(NSTEP, H, BS)  # [s,h,b]
        ao = np.moveaxis(ao, (2, 0), (1, 2)).reshape(NSTEP, H, BS)

        S = np.einsum("s,shb->hb", gamma[0::2], ae) + np.einsum(
            "s,shb->hb", gamma[1::2], ao
        )
        r0 = c * BS
        shard = y1[r0 : r0 + BS].astype(np.float64)  # [BS, D]
        y_fin = c_y * shard + (W2_ @ S).T + c_b * b2_[None, :]
        ptr = np.einsum("h,shb->b", cvec, ae ** 2)
        i_fin = HSTEP * (NSTEP * sum_c - ptr)
        out[r0 : r0 + BS, :D] = y_fin.astype(np.float32)
        out[r0 : r0 + BS, D] = i_fin.astype(np.float32)

    if _trace:
        return out.astype(in_dtype, copy=False), res
    return out.astype(in_dtype, copy=False)


# revision 13
# speedup vs baseline: 1.0151x; 1.0151x over previous
"""Trainium2 Bass kernel for the CNF reversible backward solve.

Architecture (v7): the wall-clock is bound by the serial per-step chain
  tanh_even -> Z-matmuls -> tanh_odd -> V-matmuls
so both state banks are eternal PSUM accumulators and both tanh evals run
as ONE custom DVE instruction each (clamped odd-quintic minimax fit of
tanh, max pointwise err 1.4e-2, end-to-end rel err ~6e-3 -- validated in
fp64 simulation against the reference):

  out = clamp(x*(C0 + imm2*x^2 + Src1*x^4), -C1, +C1)

The Y-carry is eliminated by the substitution V_s = l^s * Yhat_s, making V
a pure accumulator:
  V_{s+1} = V_s + l^s*Mz a_o(s) + l^s(l-1)*Zb_{s+1} + l^s*D(s)
with a_e(s) = tanh(inv_l^{s+1} V_s) (input scale folded into the DVE-op
coefficients). The l^s factors go into 8 epoch-relative weight copies
(mztV[m] = l^m Mz, m = s mod 8) plus an in-place DVE rescale of the V bank
by inv_l^8 every 8 steps. The Zb coupling rides q = kappa_m*Zb computed on
the otherwise-idle Scalar engine (Copy-activation with scale) and injected
via an identity matmul; per-step bias deltas use baseline-style rank-2
bf16 matmuls.

Sharding: data-parallel, B=256 -> 32 samples on each of 8 cores.
Activations stream to DRAM; outputs are assembled host-side from
gamma-weighted activation sums (exact coefficient recursions in fp64).
"""

import numpy as np
import ml_dtypes
from contextlib import ExitStack

import concourse.bass as bass
import concourse.tile as tile
from concourse import bacc, mybir
from concourse import dve_ops as _dops
from concourse.dve_ops import DveOp
from concourse.dve_spec import (
    Spec, Src0, Src1, C0, C1, C2, Zero, minn, maxx, sq, lower, _has_src1,
)
from concourse.dve_uop import DveOpSpec
from concourse.bass_utils import run_bass_kernel_spmd

# Problem constants (hardcoded per contract)
NCORES = 8
B, D, H = 256, 64, 256
NSTEP = 64
HSTEP = 1.0 / NSTEP
LCOUP = 0.999
INVL = 1.0 / LCOUP
BS = B // NCORES  # 32 samples per core
NBLK = H // 128  # 2 h-blocks
FREE = NBLK * BS  # 64: free size of H-space tiles, layout (blk, sample)
EPOCH = 16
DMA_CHUNKS = 8
CSTEPS = NSTEP // DMA_CHUNKS  # 16 steps per out-DMA chunk
CCOLS = CSTEPS * FREE
ACOLS = NSTEP * FREE

F32 = mybir.dt.float32
BF16 = mybir.dt.bfloat16
BF16NP = ml_dtypes.bfloat16

# clamped odd-quintic minimax fit of tanh on [0, 6]
A0 = 0.9535417
A1 = -0.20116429
A2 = 0.02062697
CC = 0.98560722

SHARED_INPUTS = [
    "w1tb", "mzt", "mztv", "mztq", "ib16", "dbyv", "f32blob", "bf16blob",
    "a2tab",
]


# --- custom DVE op ---------------------------------------------------------

def _tanhq_ref(in0, in1, s0, s1, imm2):
    x = in0.astype(np.float32)
    u = x * x
    r = x * (np.float32(s0) + np.float32(imm2) * u + in1.astype(np.float32) * u * u)
    return np.clip(r, -np.float32(s1), np.float32(s1))


def _register_tanhq():
    name = "TANH_POLY_ANT"
    if name in _dops._SUB_OPCODE_FOR_NAME:
        return next(op for op in _dops.OPS if op.name == name)
    u = sq(Src0)
    p = (u * Src1 + C2) * u + C0
    body = maxx(minn(p * Src0, C1), Zero - C1)
    spec = Spec(body=body, reference=_tanhq_ref)
    uops = lower(spec, ver="v3")
    sha = DveOpSpec(name=name, opcode=0, uops=uops, rd1_en=_has_src1(spec)).sha("v3")
    op = DveOp(name, spec, subdim=False, uops_sha={"v3": sha})
    row = max(_dops._SUB_OPCODE_FOR_NAME.values()) + 1
    assert row < 0x20
    _dops.OPS.append(op)
    _dops._SUB_OPCODE_FOR_NAME[name] = row
    _dops.CUSTOM_DVE_SPECS[name] = op.spec
    return op


TANHQ = _register_tanhq()


def np_tanhq(x):
    x = np.asarray(x, dtype=np.float64)
    r = x * (A0 + A1 * x * x + A2 * x ** 4)
    return np.clip(r, -CC, CC)


# --- host-side tables ------------------------------------------------------

def _coefficients():
    """Exact fp64 scalar recursions for the output-extraction weights
    (identical to the reference recursion; device streams a_e, a_o)."""
    gamma = np.zeros(2 * NSTEP)
    la = np.zeros(2 * NSTEP)
    alpha_y = alpha_z = 1.0
    nu_y = nu_z = 0.0
    for s in range(NSTEP):
        la[2 * s] += -HSTEP
        nu_z += -HSTEP
        gamma *= INVL
        alpha_y *= INVL
        nu_y *= INVL
        gamma += (1.0 - INVL) * la
        alpha_y += (1.0 - INVL) * alpha_z
        nu_y += (1.0 - INVL) * nu_z
        gamma[2 * s + 1] += -INVL * HSTEP
        nu_y += -INVL * HSTEP
    return gamma, alpha_y, nu_y


def _host_tables(W1, b1, u1, W2, b2):
    W1 = W1.astype(np.float64)
    W2 = W2.astype(np.float64)
    b1 = b1.astype(np.float64)
    u1 = u1.astype(np.float64)
    b2 = b2.astype(np.float64)
    l = LCOUP

    Mz = -HSTEP * (W1 @ W2)  # [H, H]
    W1b2 = W1 @ b2

    def be(s):
        return b1 + (1.0 - s * HSTEP) * u1

    def bp(s):
        return b1 + (1.0 - (s + 1) * HSTEP) * u1 - (s + 1) * HSTEP * W1b2

    # mzt_pack[p, (k*NBLK+j)*128 + q] = Mz[128j+q, 128k+p]
    MzT = Mz.T
    def pack(M):
        out = np.zeros((128, NBLK * NBLK * 128))
        for k in range(NBLK):
            for j in range(NBLK):
                out[:, (k * NBLK + j) * 128 : (k * NBLK + j + 1) * 128] = M[
                    128 * k : 128 * k + 128, 128 * j : 128 * j + 128
                ]
        return out

    mzt_pack = pack(MzT)
    # slot m scales l^m; the last slot of each epoch additionally folds the
    # epoch rescale inv_l^EPOCH (the V bank is rescaled mid-step, before its
    # own step's contributions land)
    def mscale(m):
        sc = l ** m
        if m == EPOCH - 1:
            sc *= (1.0 / l) ** EPOCH
        return sc
    mztv = np.concatenate([mscale(m) * mzt_pack for m in range(EPOCH)], axis=1)
    mztq = (l - 1.0) * mztv

    # rank-2 bias tables
    dzc = -HSTEP * u1 - HSTEP * W1b2  # constant Zb delta (s>=1)
    dzcb = np.zeros((2, 128))
    for k in range(NBLK):
        dzcb[k, :] = dzc[128 * k : 128 * k + 128]

    dzc_ = dzc
    dbyv = np.zeros((2, NSTEP * 128))
    for s in range(NSTEP - 1):
        m = s % EPOCH
        Ds = -be(s) - (l - 1.0) * bp(s) - HSTEP * W1b2 + l * be(s + 1)
        sc = l ** m
        if m == EPOCH - 1:
            sc *= (1.0 / l) ** EPOCH
        v = sc * Ds
        if s >= 1:
            v = v + sc * (l - 1.0) * dzc_
        for k in range(NBLK):
            dbyv[k, s * 128 : (s + 1) * 128] = v[128 * k : 128 * k + 128]

    # init biases (fp32 rank-2)
    zb0 = np.zeros((2, 128))
    vb0 = np.zeros((2, 128))
    for k in range(NBLK):
        zb0[k, :] = bp(0)[128 * k : 128 * k + 128]
        vb0[k, :] = (l * be(0))[128 * k : 128 * k + 128]

    ind = np.zeros((2, FREE))
    for k in range(NBLK):
        ind[k, k * BS : (k + 1) * BS] = 1.0

    # a2 coefficient tiles (full-width: the [P,1]-broadcast in1 mode is
    # broken on HW): block m in [0,EPOCH) = even-eval A2*sigma_m^5, block
    # EPOCH = odd-eval plain A2
    a2tab = np.zeros((128, (EPOCH + 1) * FREE))
    for m in range(EPOCH):
        a2tab[:, m * FREE : (m + 1) * FREE] = A2 * (INVL ** (m + 1)) ** 5
    a2tab[:, EPOCH * FREE :] = A2

    f32blob = np.concatenate([zb0, vb0, ind], axis=1)  # [2, 256+64]
    bf16blob = np.concatenate([dzcb, ind], axis=1)  # [2, 128+64]
    w1tb = np.concatenate([W1.T, l * W1.T], axis=1)  # [64, 512]
    return dict(
        mzt=mzt_pack.astype(BF16NP),
        mztv=mztv.astype(BF16NP),
        mztq=mztq.astype(BF16NP),
        ib16=np.eye(128).astype(BF16NP),
        dbyv=dbyv.astype(BF16NP),
        f32blob=f32blob.astype(np.float32),
        bf16blob=bf16blob.astype(BF16NP),
        a2tab=a2tab.astype(np.float32),
        w1tb=w1tb.astype(np.float32),
    )


# --- device kernel ---------------------------------------------------------

def _build_kernel():
    nc = bacc.Bacc("TRN2", target_bir_lowering=False, debug=False)

    y1t_d = nc.dram_tensor("y1t", [D, BS], F32, kind="ExternalInput").ap()
    w1tb_d = nc.dram_tensor("w1tb", [D, 2 * H], F32, kind="ExternalInput").ap()
    mzt_d = nc.dram_tensor("mzt", [128, NBLK * NBLK * 128], BF16, kind="ExternalInput").ap()
    mztv_d = nc.dram_tensor("mztv", [128, EPOCH * NBLK * NBLK * 128], BF16, kind="ExternalInput").ap()
    mztq_d = nc.dram_tensor("mztq", [128, EPOCH * NBLK * NBLK * 128], BF16, kind="ExternalInput").ap()
    ib16_d = nc.dram_tensor("ib16", [128, 128], BF16, kind="ExternalInput").ap()
    dbyv_d = nc.dram_tensor("dbyv", [2, NSTEP * 128], BF16, kind="ExternalInput").ap()
    f32blob_d = nc.dram_tensor("f32blob", [2, 256 + FREE], F32, kind="ExternalInput").ap()
    bf16blob_d = nc.dram_tensor("bf16blob", [2, 128 + FREE], BF16, kind="ExternalInput").ap()
    a2tab_d = nc.dram_tensor("a2tab", [128, (EPOCH + 1) * FREE], F32, kind="ExternalInput").ap()

    ae_out_d = nc.dram_tensor("ae_out", [128, ACOLS], BF16, kind="ExternalOutput").ap()
    ao_out_d = nc.dram_tensor("ao_out", [128, ACOLS], BF16, kind="ExternalOutput").ap()

    with tile.TileContext(nc) as tc, ExitStack() as ctx:
        consts = ctx.enter_context(tc.tile_pool(name="consts", bufs=1))
        zpool = ctx.enter_context(tc.tile_pool(name="zps", bufs=1, space="PSUM"))
        vpool = ctx.enter_context(tc.tile_pool(name="vps", bufs=1, space="PSUM"))
        qpool = ctx.enter_context(tc.tile_pool(name="qtmp", bufs=2))

        def cload(name, shape, dt, dram):
            t = consts.tile(shape, dt, tag=name, name=name)
            nc.sync.dma_start(t[:], dram)
            return t

        # init-critical loads split across two DMA queues (sync + gpsimd)
        y1t = consts.tile([D, BS], F32, tag="y1t", name="y1t")
        nc.sync.dma_start(y1t[:], y1t_d)
        w1tb = consts.tile([D, 2 * H], F32, tag="w1tb", name="w1tb")
        nc.gpsimd.dma_start(w1tb[:], w1tb_d)
        f32blob = consts.tile([2, 256 + FREE], F32, tag="f32blob", name="f32blob")
        nc.sync.dma_start(f32blob[:], f32blob_d)
        a2tab = consts.tile([128, (EPOCH + 1) * FREE], F32, tag="a2tab", name="a2tab")
        nc.gpsimd.dma_start(a2tab[:], a2tab_d)
        bf16blob = consts.tile([2, 128 + FREE], BF16, tag="bf16blob", name="bf16blob")
        nc.sync.dma_start(bf16blob[:], bf16blob_d)
        mzt = consts.tile([128, NBLK * NBLK * 128], BF16, tag="mzt", name="mzt")
        nc.gpsimd.dma_start(mzt[:], mzt_d)
        ib16 = consts.tile([128, 128], BF16, tag="ib16", name="ib16")
        nc.sync.dma_start(ib16[:], ib16_d)
        dbyv = consts.tile([2, NSTEP * 128], BF16, tag="dbyv", name="dbyv")
        nc.gpsimd.dma_start(dbyv[:], dbyv_d)

        # big per-epoch weight packs streamed in epoch order (epoch m is
        # first needed at step 8m)
        EB = NBLK * NBLK * 128
        mztv = consts.tile([128, EPOCH * EB], BF16, tag="mztv", name="mztv")
        mztq = consts.tile([128, EPOCH * EB], BF16, tag="mztq", name="mztq")
        for m in range(EPOCH):
            nc.sync.dma_start(mztv[:, m * EB : (m + 1) * EB], mztv_d[:, m * EB : (m + 1) * EB])
            nc.gpsimd.dma_start(mztq[:, m * EB : (m + 1) * EB], mztq_d[:, m * EB : (m + 1) * EB])

        abuf_e = [
            consts.tile([128, CCOLS], BF16, tag=f"abe{c}", name=f"abe{c}")
            for c in range(DMA_CHUNKS)
        ]
        abuf_o = [
            consts.tile([128, CCOLS], BF16, tag=f"abo{c}", name=f"abo{c}")
            for c in range(DMA_CHUNKS)
        ]

        def mzt_blk(k, j):
            base = (k * NBLK + j) * 128
            return mzt[:, base : base + 128]

        def mztv_blk(m, k, j):
            base = (m * NBLK * NBLK + k * NBLK + j) * 128
            return mztv[:, base : base + 128]

        def mztq_blk(m, k, j):
            base = (m * NBLK * NBLK + k * NBLK + j) * 128
            return mztq[:, base : base + 128]

        # --- init: V = l*(W1 y + be(0)) first (gates the first tanh);
        # Zb = W1 y + bp(0) second (only gates the first Z-phase) ---
        vb = vpool.tile([128, FREE], F32, tag="vb", name="vb")
        for j in range(NBLK):
            nc.tensor.matmul(
                vb[:, j * BS : (j + 1) * BS],
                w1tb[:, H + 128 * j : H + 128 * j + 128],
                y1t[:],
                start=(j == 0),
                stop=False,
            )
        nc.tensor.matmul(vb[:], f32blob[:, 128:256], f32blob[:, 256 : 256 + FREE], start=False, stop=True)

        zb = zpool.tile([128, FREE], F32, tag="zb", name="zb")
        for j in range(NBLK):
            nc.tensor.matmul(
                zb[:, j * BS : (j + 1) * BS],
                w1tb[:, 128 * j : 128 * j + 128],
                y1t[:],
                start=(j == 0),
                stop=False,
            )
        nc.tensor.matmul(zb[:], f32blob[:, 0:128], f32blob[:, 256 : 256 + FREE], start=False, stop=True)

        for s in range(NSTEP):
            last = s == NSTEP - 1
            m = s % EPOCH
            chunk, cstep = divmod(s, CSTEPS)
            ecol = cstep * FREE
            sigma = INVL ** (m + 1)

            # q' = kappa_m * Zb_pre on the scalar engine (pre-step Zb state;
            # runs in the tanh-even window, different PSUM bank)
            epoch_end = (m == EPOCH - 1) and not last
            qsc = (LCOUP ** m) * (LCOUP - 1.0)
            if epoch_end:
                qsc *= INVL ** EPOCH
            if not last:
                q_t = qpool.tile([128, FREE], BF16, tag="q", name=f"q{s}")
                nc.scalar.activation(
                    q_t[:], zb[:], mybir.ActivationFunctionType.Copy,
                    scale=float(qsc),
                )

            # even eval: a_e = tanhq(sigma * V)
            a_e = abuf_e[chunk][:, ecol : ecol + FREE]
            nc.vector._custom_dve(
                TANHQ, out=a_e, in0=vb[:], in1=a2tab[:, m * FREE : (m + 1) * FREE],
                s0=A0 * sigma, s1=CC, imm2=A1 * sigma ** 3,
            )

            # Zb += dzc (s>=1) + Mz a_e
            if s > 0:
                nc.tensor.matmul(zb[:], bf16blob[:, 0:128], bf16blob[:, 128 : 128 + FREE], start=False,
                                 stop=False, skip_group_check=True)
            for j in range(NBLK):
                for k in range(NBLK):
                    nc.tensor.matmul(
                        zb[:, j * BS : (j + 1) * BS],
                        mzt_blk(k, j),
                        a_e[:, k * BS : (k + 1) * BS],
                        start=False,
                        stop=False,
                        skip_group_check=True,
                    )

            if epoch_end:
                # epoch rescale, overlapped with the Z-matmul phase: V was
                # just read by tanh-even; this step's own V-contributions
                # carry the inv_l^EPOCH fold in their scales
                nc.vector.tensor_scalar_mul(vb[:], vb[:], float(INVL ** EPOCH))

            if not last:
                # a_e-dependent V terms (run during tanh-odd): dbyv const,
                # q' inject, kappa_m*Mz @ a_e
                nc.tensor.matmul(vb[:], dbyv[:, s * 128 : (s + 1) * 128], bf16blob[:, 128 : 128 + FREE],
                                 start=False, stop=False, skip_group_check=True)
                nc.tensor.matmul(vb[:], ib16[:], q_t[:], start=False,
                                 stop=False, skip_group_check=True)
                for j in range(NBLK):
                    for k in range(NBLK):
                        nc.tensor.matmul(
                            vb[:, j * BS : (j + 1) * BS],
                            mztq_blk(m, k, j),
                            a_e[:, k * BS : (k + 1) * BS],
                            start=False,
                            stop=False,
                            skip_group_check=True,
                        )

            # odd eval: a_o = tanhq(Zb)
            a_o = abuf_o[chunk][:, ecol : ecol + FREE]
            nc.vector._custom_dve(
                TANHQ, out=a_o, in0=zb[:],
                in1=a2tab[:, EPOCH * FREE : (EPOCH + 1) * FREE],
                s0=A0, s1=CC, imm2=A1,
            )

            if not last:
                # V += l^m Mz a_o
                for j in range(NBLK):
                    for k in range(NBLK):
                        nc.tensor.matmul(
                            vb[:, j * BS : (j + 1) * BS],
                            mztv_blk(m, k, j),
                            a_o[:, k * BS : (k + 1) * BS],
                            start=False,
                            stop=False,
                            skip_group_check=True,
                        )


            if (s + 1) % CSTEPS == 0:
                c0 = chunk * CCOLS
                nc.sync.dma_start(ae_out_d[:, c0 : c0 + CCOLS], abuf_e[chunk][:])
                nc.sync.dma_start(ao_out_d[:, c0 : c0 + CCOLS], abuf_o[chunk][:])

    nc.compile()
    return nc


_CACHE = {}


def _get_kernel():
    if "nc" not in _CACHE:
        _CACHE["nc"] = _build_kernel()
    return _CACHE["nc"]


def kernel(y1, W1, b1, u1, W2, b2, _trace=False, _trace_kwargs=None):
    y1 = np.asarray(y1)
    in_dtype = y1.dtype
    W1_ = np.asarray(W1, dtype=np.float64)
    W2_ = np.asarray(W2, dtype=np.float64)
    b2_ = np.asarray(b2, dtype=np.float64)
    tabs = _host_tables(
        np.asarray(W1), np.asarray(b1), np.asarray(u1), np.asarray(W2), np.asarray(b2)
    )

    nc = _get_kernel()

    shared = {k: tabs[k] for k in SHARED_INPUTS}
    in_maps = []
    for c in range(NCORES):
        mmap = dict(shared)
        shard = y1[c * BS : (c + 1) * BS].astype(np.float32)  # [BS, D]
        mmap["y1t"] = np.ascontiguousarray(shard.T)  # [D, BS]
        in_maps.append(mmap)

    kw = {}
    if _trace:
        kw["trace"] = True
        if _trace_kwargs:
            kw.update(_trace_kwargs)
    res = run_bass_kernel_spmd(nc, in_maps, core_ids=list(range(NCORES)), **kw)

    # --- host-side output extraction ---
    gamma, c_y, c_b = _coefficients()
    cvec = np.sum(W1_ * W2_.T, axis=1)  # diag(W1@W2)
    sum_c = float(np.sum(cvec))

    out = np.zeros((B, D + 1), dtype=np.float32)
    for c in range(NCORES):
        ae = np.asarray(res.results[c]["ae_out"]).astype(np.float64)
        ao = np.asarray(res.results[c]["ao_out"]).astype(np.float64)
        ae = ae.reshape(128, NSTEP, NBLK, BS)  # [p, s, blk, b]
        ao = ao.reshape(128, NSTEP, NBLK, BS)
        ae = np.moveaxis(ae, (2, 0), (1, 2)).reshape(NSTEP, H, BS)  # [s,h,b]
        ao = np.moveaxis(ao, (2, 0), (1, 2)).reshape(NSTEP, H, BS)

        S = np.einsum("s,shb->hb", gamma[0::2], ae) + np.einsum(
            "s,shb->hb", gamma[1::2], ao
        )
        r0 = c * BS
        shard = y1[r0 : r0 + BS].astype(np.float64)  # [BS, D]
        y_fin = c_y * shard + (W2_ @ S).T + c_b * b2_[None, :]
        ptr = np.einsum("h,shb->b", cvec, ae ** 2)
        i_fin = HSTEP * (NSTEP * sum_c - ptr)
        out[r0 : r0 + BS, :D] = y_fin.astype(np.float32)
        out[r0 : r0 + BS, D] = i_fin.astype(np.float32)

    if _trace:
        return out.astype(in_dtype, copy=False), res
    return out.astype(in_dtype, copy=False)
